# revision 1
# baseline (speedup 1.0000x reference)
"""Trainium2 Bass kernel for nn_CustomNetwork_37031208026716.

Network: 32 layers of (depth-1 butterfly rotation + interleave permutation +
smooth-bend activation) on X[65536, 512] fp32.

Strategy:
  * Pure data parallel over 8 cores (batch split, 8192 rows/core).
  * On-chip layout: width (512) on partitions as 4 tiles of 128; batch on the
    free axis.  Shards are pre-transposed on the host so DMAs are contiguous.
  * The per-layer interleave permutation is never materialized.  Instead the
    computation runs in "conjugated" coordinates: physical index w at layer l
    corresponds to logical index pi_l(w) (pi_l = P^-l for the rotate
    permutation P).  The butterfly then pairs physical index w with w^delta_l,
    delta_l = 2^((8-l) mod 9), and all per-width parameters are gathered by
    pi on the host.
  * Layers with delta >= 128 pair whole partition-tiles: computed with DVE
    tensor_scalar + scalar_tensor_tensor (per-partition scalar coefficients).
  * Layers with delta < 128 pair partitions within a tile: computed on the
    TensorEngine as an exact fp32 matmul with a 2-nonzeros-per-row 128x128
    butterfly matrix (permutation folded into the matrix).  The additive bias
    is propagated as a host-tracked offset beta and folded into the ACT pass
    (activation computes func(scale*x + bias) with per-partition APs).
  * Activation: u = n - bb;  y = u + cc*sqrt(u^2 + 1/k)
      A2: v = Square(ccabs*n + (-ccabs*bb))   [= cc^2 u^2]      (ScalarE)
      A3: w = Sqrt(v + cc^2/k)                                  (ScalarE)
      P5: y = (w * sgn) + n                                     (VectorE)
    with y stored offset by beta (exactly bb), corrected at the chain end.
"""

import numpy as np

BATCH = 65536
W = 512
HALF = 256
DEPTH = 32
NBITS = 9
NCORES = 8
NB = BATCH // NCORES          # batch rows per core
CH = 2048                     # batch columns per on-chip chunk
NTILE = 4                     # width tiles of 128 partitions
MM_F32R = False               # float32r needs rounded producers; too lossy
MIXED_DELTAS = (64, 32)       # deltas with tiles g0,g1 on PE and g2,g3 on DVE+DMA

_P_ARR = np.array([(w >> 1) | ((w & 1) << 8) for w in range(W)], dtype=np.int64)


def _invert(p):
    inv = np.empty_like(p)
    inv[p] = np.arange(len(p))
    return inv


def _build_perms():
    pinv = _invert(_P_ARR)
    pis = [np.arange(W)]
    for l in range(DEPTH):
        pis.append(pinv[pis[l]])
    return pis


def _deltas():
    return [1 << ((8 - l) % NBITS) for l in range(DEPTH)]


def _pe_gs(delta):
    """Which width-tiles run the butterfly on the TensorEngine for a layer."""
    if delta >= 128:
        return ()
    if delta in MIXED_DELTAS:
        return (0, 1)
    return (0, 1, 2, 3)


def _pe_lg():
    return [(l, g) for l, d in enumerate(_deltas()) for g in _pe_gs(d)]


def host_precompute(thetas, biases, slopes1, slopes2, curvatures):
    """Returns (layers list of dicts, beta_final[W], out_perm[W])."""
    pis = _build_perms()
    thetas = thetas.astype(np.float64)
    c_all = np.cos(thetas)
    s_all = np.sin(thetas)
    m1 = np.exp(slopes1.astype(np.float64))
    m2 = np.exp(slopes2.astype(np.float64))
    a_all = (m1 + m2) * 0.5
    cc_all = (m2 - m1) / (2.0 * a_all)
    b_all = np.sinh(biases.astype(np.float64))
    ik_all = np.exp(-curvatures.astype(np.float64))

    layers = []
    beta = np.zeros(W)
    idx_w = np.arange(W)
    for l in range(DEPTH):
        pi_l, pi_n = pis[l], pis[l + 1]
        delta = 1 << ((8 - l) % NBITS)
        V = pi_l
        idx = V % HALF
        cA = c_all[l][idx]
        sA = np.where(V < HALF, s_all[l][idx], -s_all[l][idx])
        a_eff = a_all[l][pi_n]
        b_eff = b_all[l][pi_n]
        cc = cc_all[l][pi_n]
        ik = ik_all[l][pi_n]
        cAp = cA * a_eff
        sAp = sA * a_eff
        bb = cAp * beta + sAp * beta[idx_w ^ delta] + b_eff
        sgn = np.sign(cc)
        sgn[sgn == 0] = 1.0
        pe_mask = np.zeros(W, dtype=bool)
        for g in _pe_gs(delta):
            pe_mask[g * 128:(g + 1) * 128] = True
        ent = dict(
            delta=delta, cAp=cAp, sAp=sAp, bb=bb,
            ccabs=np.abs(cc), sgn=sgn, cc2=cc * cc, nbb=-bb,
            a2bias=np.where(pe_mask, -np.abs(cc) * bb, 0.0),
            a3bias=cc * cc * ik,
        )
        beta = np.where(pe_mask, bb, 0.0)
        layers.append(ent)
    return layers, beta, _invert(pis[DEPTH])


_KINDS = ("cAp", "sAp", "bb", "ccabs", "a2bias", "a3bias", "sgn", "cc2", "nbb")
PRM_COLS = (len(_KINDS) + 1) * 128   # last block: beta_final in cols [7*128 .. 7*128+4)


def pack_params(layers, beta_final):
    prm = np.zeros((128, PRM_COLS), dtype=np.float32)
    for l, ent in enumerate(layers):
        for k, kind in enumerate(_KINDS):
            vec = ent[kind]
            for g in range(NTILE):
                prm[:, k * 128 + l * NTILE + g] = vec[g * 128:(g + 1) * 128]
    for g in range(NTILE):
        prm[:, len(_KINDS) * 128 + g] = beta_final[g * 128:(g + 1) * 128]
    return prm


def pack_weights(layers):
    """Butterfly matrices for delta<128 layers, as lhsT tiles packed into a
    [128, n_pe*4*128] fp32 array (lhsT[q, p] = A[p, q], psum = lhsT.T @ x)."""
    lg = _pe_lg()
    wts = np.zeros((128, len(lg) * 128), dtype=np.float32)
    p128 = np.arange(128)
    for j, (l, g) in enumerate(lg):
        ent = layers[l]
        delta = ent["delta"]
        w = g * 128 + p128
        A = np.zeros((128, 128), dtype=np.float64)
        A[p128, p128] = ent["cAp"][w]
        A[p128, p128 ^ delta] = ent["sAp"][w]
        wts[:, j * 128:(j + 1) * 128] = A.T.astype(np.float32)
    return wts, lg


def build_nc(nb=NB, ch=CH):
    """Build the Bass module for one core (SPMD: all cores identical)."""
    from concourse import bacc, bass, mybir
    from concourse.tile import TileContext

    f32 = mybir.dt.float32
    AT = mybir.ActivationFunctionType
    OP = mybir.AluOpType

    layers_meta = [(l, d) for l, d in enumerate(_deltas())]
    lg = _pe_lg()
    n_pe4 = len(lg)
    widx = {key: j for j, key in enumerate(lg)}

    nc = bacc.Bacc(target_bir_lowering=False, debug=False)
    xt = nc.declare_dram_parameter("xt", [W, nb], f32, isOutput=False)
    prm_d = nc.declare_dram_parameter("prm", [128, PRM_COLS], f32, isOutput=False)
    wts_d = nc.declare_dram_parameter("wts", [128, n_pe4 * 128], f32,
                                      isOutput=False)
    out_d = nc.declare_dram_parameter("out", [W, nb], f32, isOutput=True)

    KIDX = {k: i * 128 for i, k in enumerate(_KINDS)}
    BETA0 = len(_KINDS) * 128

    nchunk = nb // ch
    mmn = min(512, ch)            # moving free-dim per matmul (1 PSUM bank)

    with TileContext(nc) as tc:
        with (
            tc.tile_pool(name="const", bufs=1) as cpool,
            tc.tile_pool(name="x", bufs=1) as xpool,
            tc.tile_pool(name="ps", bufs=2, space="PSUM") as pspool,
            tc.tile_pool(name="tv", bufs=3) as vpool,
            tc.tile_pool(name="tw", bufs=3) as wpool,
            tc.tile_pool(name="tu", bufs=3) as upool,
            tc.tile_pool(name="sh", bufs=3) as shpool,
        ):
            prm = cpool.tile([128, PRM_COLS], f32, tag="prm", name="prm")
            nc.sync.dma_start(out=prm, in_=prm_d[:, :])
            wsb = cpool.tile([128, n_pe4 * 128], f32, tag="wts", name="wsb")
            nc.sync.dma_start(out=wsb, in_=wts_d[:, :])

            def pcol(kind, l, g):
                c = KIDX[kind] + l * NTILE + g
                return prm[:, c:c + 1]

            xt_r = xt.rearrange("(g p) b -> p g b", g=NTILE)
            out_r = out_d.rearrange("(g p) b -> p g b", g=NTILE)

            SUPER = 2                 # chunks resident per superpass
            for sp in range(nchunk // SUPER):
                cs = list(range(sp * SUPER, (sp + 1) * SUPER))
                xts = {}
                for ci, c in enumerate(cs):
                    for g in range(NTILE):
                        xts[(c, g)] = xpool.tile(
                            [128, ch], f32, tag=f"x{ci}_{g}", name=f"xt{ci}_{g}")
                        nc.sync.dma_start(
                            out=xts[(c, g)],
                            in_=xt_r[:, g, c * ch:(c + 1) * ch])

                for l, delta in layers_meta:
                    pe_gs = _pe_gs(delta)
                    dve_gs = [g for g in range(NTILE) if g not in pe_gs]
                    partners = {}
                    if dve_gs and delta < 128:
                        for c in cs:
                            for g in dve_gs:
                                sh = shpool.tile([128, ch], f32, tag="sh",
                                                 name="sh")
                                for b0 in range(0, 128, 2 * delta):
                                    nc.sync.dma_start(
                                        out=sh[b0:b0 + delta, :],
                                        in_=xts[(c, g)][b0 + delta:b0 + 2 * delta, :])
                                    nc.sync.dma_start(
                                        out=sh[b0 + delta:b0 + 2 * delta, :],
                                        in_=xts[(c, g)][b0:b0 + delta, :])
                                partners[(c, g)] = sh
                    elif dve_gs:
                        t = delta >> 7
                        for c in cs:
                            for g in dve_gs:
                                partners[(c, g)] = xts[(c, g ^ t)]
                    for c in cs:
                        us = {}
                        for g in dve_gs:
                            u = upool.tile([128, ch], f32, tag="u", name="u")
                            nc.vector.tensor_scalar(
                                u, xts[(c, g)], pcol("cAp", l, g),
                                pcol("bb", l, g), OP.mult, OP.subtract)
                            nc.vector.scalar_tensor_tensor(
                                u, partners[(c, g)], pcol("sAp", l, g), u,
                                OP.mult, OP.add)
                            us[g] = u
                        for g in pe_gs:
                            ps = pspool.tile([128, ch], f32, tag="ps", name="ps")
                            lhsT = wsb[:, widx[(l, g)] * 128:
                                       (widx[(l, g)] + 1) * 128]
                            for m0 in range(0, ch, mmn):
                                nc.tensor.matmul(
                                    ps[:, m0:m0 + mmn], lhsT,
                                    xts[(c, g)][:, m0:m0 + mmn],
                                    start=True, stop=True)
                            v = vpool.tile([128, ch], f32, tag="v", name="v")
                            nc.scalar.activation(
                                v, ps, AT.Square,
                                bias=pcol("a2bias", l, g),
                                scale=pcol("ccabs", l, g))
                            wt = wpool.tile([128, ch], f32, tag="w",
                                            name="wtile")
                            nc.scalar.activation(
                                wt, v, AT.Sqrt,
                                bias=pcol("a3bias", l, g), scale=1.0)
                            nc.vector.scalar_tensor_tensor(
                                xts[(c, g)], wt, pcol("sgn", l, g), ps,
                                OP.mult, OP.add)
                        for g in dve_gs:
                            u = us[g]
                            v = vpool.tile([128, ch], f32, tag="v", name="v")
                            nc.scalar.activation(
                                v, u, AT.Square, bias=0.0,
                                scale=pcol("ccabs", l, g))
                            wt = wpool.tile([128, ch], f32, tag="w",
                                            name="wtile")
                            nc.scalar.activation(
                                wt, v, AT.Sqrt,
                                bias=pcol("a3bias", l, g), scale=1.0)
                            nc.vector.scalar_tensor_tensor(
                                xts[(c, g)], wt, pcol("sgn", l, g), u,
                                OP.mult, OP.add)

                for c in cs:
                    for g in range(NTILE):
                        nc.vector.tensor_scalar(
                            xts[(c, g)], xts[(c, g)],
                            prm[:, BETA0 + g:BETA0 + g + 1], None,
                            OP.subtract)
                        nc.sync.dma_start(
                            out=out_r[:, g, c * ch:(c + 1) * ch],
                            in_=xts[(c, g)])
    nc.compile()
    return nc


_NC_CACHE = {}

# test-harness knobs (not used by the grading path)
TRACE = False
TRACE_KWARGS = {}
LAST_RESULTS = None


def _get_nc(nb, ch):
    key = (nb, ch)
    if key not in _NC_CACHE:
        _NC_CACHE[key] = build_nc(nb, ch)
    return _NC_CACHE[key]


def kernel(X, thetas, biases, slopes1, slopes2, curvatures):
    global LAST_RESULTS
    from concourse.bass_utils import run_bass_kernel_spmd

    X = np.asarray(X)
    layers, beta_final, out_perm = host_precompute(
        np.asarray(thetas), np.asarray(biases), np.asarray(slopes1),
        np.asarray(slopes2), np.asarray(curvatures))
    prm = pack_params(layers, beta_final)
    wts, _ = pack_weights(layers)

    nc = _get_nc(NB, CH)
    in_maps = []
    for cid in range(NCORES):
        shard = np.ascontiguousarray(X[cid * NB:(cid + 1) * NB, :].T)
        in_maps.append({"xt": shard, "prm": prm, "wts": wts})

    res = run_bass_kernel_spmd(nc, in_maps, list(range(NCORES)),
                               trace=TRACE, **TRACE_KWARGS)
    LAST_RESULTS = res
    out = np.empty((BATCH, W), dtype=np.float32)
    for cid in range(NCORES):
        o = res.results[cid]["out"]          # [512, NB] physical order
        out[cid * NB:(cid + 1) * NB, :] = o[out_perm, :].T
    return out



# revision 7
# speedup vs baseline: 1.6430x; 1.6430x over previous
"""Trainium2 Bass kernel for nn_CustomNetwork_37031208026716.

Network: 32 layers of (depth-1 butterfly rotation + interleave permutation +
smooth-bend activation y = u + cc*sqrt(u^2 + ik)) on X[65536, 512] fp32.

Strategy ("pair-compose", fp16 resident):
  * Pure data parallel over 8 cores (batch split, 8192 rows/core).
  * Width (512) on partitions as 4 tiles of 128; batch on the free axis,
    fp16 in SBUF.  Interleave permutation via conjugated coordinates:
    physical w at layer l is logical pi_l(w); butterfly pairs w with
    w^delta_l, delta_l = 2^((8-l)%9) (cross-tile when delta>=128).
  * Layer state is the PAIR (U, T): U = u (pre-bend affine value),
    T = sqrt(u^2+ik).  y = U + cc*T is only materialized (as Y, carrying a
    host-tracked additive offset) on layers feeding a cross-tile butterfly,
    so EVERY layer runs exactly two 128x128 fp16 matmul matrices:
      within-tile :  n = MU@U + MT@T      (MT folds prev layer's cc)
      cross-tile  :  n = D_s*Y_g + D_p*Y_g^  (diagonal matrices)
  * Remaining per-layer elementwise work, balanced across DVE and ACT:
      pair layers:  pull U' = n - pb  (DVE tensor_scalar from PSUM, or ACT
                    Identity from PSUM -- static split), q = U'*U' (DVE
                    tensor_tensor fp16 2x), T' = sqrt(q + ik) (ACT).
      ymat layers:  q = Square(n - pb) directly from PSUM (ACT),
                    T' = sqrt(q + ik) (ACT), Y' = cc*T' + n (DVE stt from
                    PSUM; carries offset pb which the host folds into the
                    next layer's biases).
  * Host casts fp32->fp16 for input/output transfers (halves HBM traffic).
"""

import numpy as np

BATCH = 65536
W = 512
HALF = 256
DEPTH = 32
NBITS = 9
NCORES = 8
NB = BATCH // NCORES          # batch rows per core
CH = 2048                     # batch columns per on-chip chunk
NTILE = 4                     # width tiles of 128 partitions
MMH = 512                     # moving free-dim per matmul (ISA cap)

CROSS = frozenset(l for l, d in enumerate(
    [1 << ((8 - l) % NBITS) for l in range(DEPTH)]) if d >= 128)
YMAT = frozenset(l for l in range(DEPTH) if (l + 1) in CROSS)

_P_ARR = np.array([(w >> 1) | ((w & 1) << 8) for w in range(W)], dtype=np.int64)


def _invert(p):
    inv = np.empty_like(p)
    inv[p] = np.arange(len(p))
    return inv


def _build_perms():
    pinv = _invert(_P_ARR)
    pis = [np.arange(W)]
    for l in range(DEPTH):
        pis.append(pinv[pis[l]])
    return pis


def _deltas():
    return [1 << ((8 - l) % NBITS) for l in range(DEPTH)]


def host_precompute(thetas, biases, slopes1, slopes2, curvatures):
    pis = _build_perms()
    thetas = thetas.astype(np.float64)
    c_all = np.cos(thetas)
    s_all = np.sin(thetas)
    m1 = np.exp(slopes1.astype(np.float64))
    m2 = np.exp(slopes2.astype(np.float64))
    a_all = (m1 + m2) * 0.5
    cc_all = (m2 - m1) / (2.0 * a_all)
    b_all = np.sinh(biases.astype(np.float64))
    ik_all = np.exp(-curvatures.astype(np.float64))

    idx_w = np.arange(W)
    layers = []
    ofs = np.zeros(W)                 # additive offset carried by Y tiles
    for l in range(DEPTH):
        pi_l, pi_n = pis[l], pis[l + 1]
        delta = 1 << ((8 - l) % NBITS)
        V = pi_l
        idx = V % HALF
        cA = c_all[l][idx]
        sA = np.where(V < HALF, s_all[l][idx], -s_all[l][idx])
        a_eff = a_all[l][pi_n]
        cAp = cA * a_eff
        sAp = sA * a_eff
        bb = b_all[l][pi_n]
        # effective pre-activation bias: subtract what the (offset) inputs
        # injected plus this layer's own bias
        pb = bb + cAp * ofs + sAp * ofs[idx_w ^ delta]
        ent = dict(delta=delta, cAp=cAp, sAp=sAp, pb=pb,
                   cc=cc_all[l][pi_n], ik=ik_all[l][pi_n])
        ofs = pb if l in YMAT else np.zeros(W)
        layers.append(ent)
    return layers, _invert(pis[DEPTH])


# ---- packing ---------------------------------------------------------------

def _wts_layout():
    out = []
    for l in range(DEPTH):
        for g in range(NTILE):
            if l in CROSS:
                out.append((l, g, "Ds"))
                out.append((l, g, "Dp"))
            else:
                out.append((l, g, "MU"))
                out.append((l, g, "MT"))
    return out


def pack_weights(layers):
    lay = _wts_layout()
    wts = np.zeros((128, len(lay) * 128), dtype=np.float16)
    p128 = np.arange(128)
    for j, (l, g, kind) in enumerate(lay):
        ent = layers[l]
        delta = ent["delta"]
        w = g * 128 + p128
        part = w ^ delta
        A = np.zeros((128, 128), dtype=np.float64)
        if kind == "Ds":
            A[p128, p128] = ent["cAp"][w]
        elif kind == "Dp":
            A[p128, p128] = ent["sAp"][w]
        elif kind == "MU":
            A[p128, p128] = ent["cAp"][w]
            A[p128, p128 ^ delta] = ent["sAp"][w]
        elif kind == "MT":
            prev = layers[l - 1]
            A[p128, p128] = ent["cAp"][w] * prev["cc"][w]
            A[p128, p128 ^ delta] = ent["sAp"][w] * prev["cc"][part]
        wts[:, j * 128:(j + 1) * 128] = A.T.astype(np.float16)
    return wts


_PKINDS = ("pb", "npb", "ik", "cc")
PRM_COLS = len(_PKINDS) * DEPTH * NTILE


def pack_params(layers):
    prm = np.zeros((128, PRM_COLS), dtype=np.float32)
    for l, ent in enumerate(layers):
        vals = dict(pb=ent["pb"], npb=-ent["pb"], ik=ent["ik"],
                    cc=ent["cc"])
        for k, kind in enumerate(_PKINDS):
            v = vals[kind]
            for g in range(NTILE):
                prm[:, (k * DEPTH + l) * NTILE + g] = v[g * 128:(g + 1) * 128]
    return prm


# ---- bass module -----------------------------------------------------------

def build_nc(nb=NB, ch=CH):
    from concourse import bacc, mybir
    from concourse.tile import TileContext

    f32 = mybir.dt.float32
    f16 = mybir.dt.float16
    AT = mybir.ActivationFunctionType
    OP = mybir.AluOpType

    deltas = _deltas()
    lay = _wts_layout()
    widx = {key: j for j, key in enumerate(lay)}

    nc = bacc.Bacc(target_bir_lowering=False, debug=False)
    xt = nc.declare_dram_parameter("xt", [W, nb], f16, isOutput=False)
    prm_d = nc.declare_dram_parameter("prm", [128, PRM_COLS], f32,
                                      isOutput=False)
    wts_d = nc.declare_dram_parameter("wts", [128, len(lay) * 128], f16,
                                      isOutput=False)
    out_d = nc.declare_dram_parameter("out", [W, nb], f16, isOutput=True)

    nchunk = nb // ch

    with TileContext(nc) as tc:
        with (
            tc.tile_pool(name="const", bufs=1) as cpool,
            tc.tile_pool(name="u", bufs=2) as upool,
            tc.tile_pool(name="t", bufs=2) as tpool,
            tc.tile_pool(name="q", bufs=3) as qpool,
            tc.tile_pool(name="y", bufs=2) as ypool,
            tc.tile_pool(name="ps", bufs=2, space="PSUM") as pspool,
        ):
            prm = cpool.tile([128, PRM_COLS], f32, tag="prm", name="prm")
            nc.sync.dma_start(out=prm, in_=prm_d[:, :])
            wsb = cpool.tile([128, len(lay) * 128], f16, tag="wts",
                             name="wsb")
            nw = len(lay) * 128
            for s in range(4):
                lo, hi = s * nw // 4, (s + 1) * nw // 4
                nc.sync.dma_start(out=wsb[:, lo:hi], in_=wts_d[:, lo:hi])

            def pcol(kind, l, g):
                k = _PKINDS.index(kind)
                c = (k * DEPTH + l) * NTILE + g
                return prm[:, c:c + 1]

            def lhs(l, g, kind):
                j = widx[(l, g, kind)]
                return wsb[:, j * 128:(j + 1) * 128]

            xt_r = xt.rearrange("(g p) b -> p g b", g=NTILE)
            out_r = out_d.rearrange("(g p) b -> p g b", g=NTILE)

            for c in range(nchunk):
                # layer-0 inputs are Y tiles (offset 0)
                Y = {}
                for g in range(NTILE):
                    Y[g] = ypool.tile([128, ch], f16, tag=f"y{g}",
                                      name=f"y{g}")
                    nc.sync.dma_start(out=Y[g],
                                      in_=xt_r[:, g, c * ch:(c + 1) * ch])

                U = {}
                T = {}
                for l in range(DEPTH):
                    delta = deltas[l]
                    Un, Tn, Yn = {}, {}, {}
                    for g in range(NTILE):
                        ps = pspool.tile([128, ch], f32, tag="ps", name="ps")
                        for h in range(0, ch, MMH):
                            sl = slice(h, h + MMH)
                            if l in CROSS:
                                gp = g ^ (delta >> 7)
                                nc.tensor.matmul(
                                    ps[:, sl], lhs(l, g, "Ds"), Y[g][:, sl],
                                    start=True, stop=False)
                                nc.tensor.matmul(
                                    ps[:, sl], lhs(l, g, "Dp"), Y[gp][:, sl],
                                    start=False, stop=True)
                            else:
                                nc.tensor.matmul(
                                    ps[:, sl], lhs(l, g, "MU"), U[g][:, sl],
                                    start=True, stop=False)
                                nc.tensor.matmul(
                                    ps[:, sl], lhs(l, g, "MT"), T[g][:, sl],
                                    start=False, stop=True)
                        if l in YMAT:
                            # q = (n - pb)^2 straight from PSUM on ACT
                            q = qpool.tile([128, ch], f16, tag="q", name="q")
                            nc.scalar.activation(
                                q, ps, AT.Square, bias=pcol("npb", l, g),
                                scale=1.0)
                            t = tpool.tile([128, ch], f16, tag=f"t{g}",
                                           name=f"t{g}")
                            nc.scalar.activation(
                                t, q, AT.Sqrt, bias=pcol("ik", l, g),
                                scale=1.0)
                            y = ypool.tile([128, ch], f16, tag=f"y{g}",
                                           name=f"y{g}")
                            nc.vector.scalar_tensor_tensor(
                                y, t, pcol("cc", l, g), ps, OP.mult, OP.add)
                            Yn[g] = y
                        else:
                            u = upool.tile([128, ch], f16, tag=f"u{g}",
                                           name=f"u{g}")
                            if (l + g) % 5 < 3:       # ~60% DVE pulls
                                nc.vector.tensor_scalar(
                                    u, ps, pcol("pb", l, g), None,
                                    OP.subtract)
                            else:
                                nc.scalar.activation(
                                    u, ps, AT.Identity,
                                    bias=pcol("npb", l, g), scale=1.0)
                            q = qpool.tile([128, ch], f16, tag="q", name="q")
                            nc.vector.tensor_tensor(q, u, u, OP.mult)
                            t = tpool.tile([128, ch], f16, tag=f"t{g}",
                                           name=f"t{g}")
                            nc.scalar.activation(
                                t, q, AT.Sqrt, bias=pcol("ik", l, g),
                                scale=1.0)
                            Un[g] = u
                            Tn[g] = t
                    if Yn:
                        Y = Yn
                    U, T = (Un or U), (Tn or T)

                for g in range(NTILE):
                    y = ypool.tile([128, ch], f16, tag=f"y{g}", name=f"o{g}")
                    nc.vector.scalar_tensor_tensor(
                        y, T[g], pcol("cc", DEPTH - 1, g), U[g],
                        OP.mult, OP.add)
                    nc.sync.dma_start(
                        out=out_r[:, g, c * ch:(c + 1) * ch], in_=y)
    nc.compile()
    return nc


_NC_CACHE = {}

TRACE = False
TRACE_KWARGS = {}
LAST_RESULTS = None


def _get_nc(nb, ch):
    key = (nb, ch)
    if key not in _NC_CACHE:
        _NC_CACHE[key] = build_nc(nb, ch)
    return _NC_CACHE[key]


def kernel(X, thetas, biases, slopes1, slopes2, curvatures):
    global LAST_RESULTS
    from concourse.bass_utils import run_bass_kernel_spmd

    X = np.asarray(X)
    layers, out_perm = host_precompute(
        np.asarray(thetas), np.asarray(biases), np.asarray(slopes1),
        np.asarray(slopes2), np.asarray(curvatures))
    prm = pack_params(layers)
    wts = pack_weights(layers)

    nc = _get_nc(NB, CH)
    in_maps = []
    for cid in range(NCORES):
        shard = np.ascontiguousarray(
            X[cid * NB:(cid + 1) * NB, :].T.astype(np.float16))
        in_maps.append({"xt": shard, "prm": prm, "wts": wts})

    res = run_bass_kernel_spmd(nc, in_maps, list(range(NCORES)),
                               trace=TRACE, **TRACE_KWARGS)
    LAST_RESULTS = res
    out = np.empty((BATCH, W), dtype=np.float32)
    for cid in range(NCORES):
        o = res.results[cid]["out"]          # [512, NB] fp16 physical order
        out[cid * NB:(cid + 1) * NB, :] = o[out_perm, :].T.astype(np.float32)
    return out


# revision 10
# speedup vs baseline: 1.6770x; 1.0207x over previous
"""Trainium2 Bass kernel for nn_CustomNetwork_37031208026716.

Network: 32 layers of (depth-1 butterfly rotation + interleave permutation +
smooth-bend activation y = u + cc*sqrt(u^2 + ik)) on X[65536, 512] fp32.

Strategy ("pair-compose", fp16 resident):
  * Pure data parallel over 8 cores (batch split, 8192 rows/core).
  * Width (512) on partitions as 4 tiles of 128; batch on the free axis,
    fp16 in SBUF.  Interleave permutation via conjugated coordinates:
    physical w at layer l is logical pi_l(w); butterfly pairs w with
    w^delta_l, delta_l = 2^((8-l)%9) (cross-tile when delta>=128).
  * Layer state is the PAIR (U, T): U = u (pre-bend affine value),
    T = sqrt(u^2+ik).  y = U + cc*T is only materialized (as Y, carrying a
    host-tracked additive offset) on layers feeding a cross-tile butterfly,
    so EVERY layer runs exactly two 128x128 fp16 matmul matrices:
      within-tile :  n = MU@U + MT@T      (MT folds prev layer's cc)
      cross-tile  :  n = D_s*Y_g + D_p*Y_g^  (diagonal matrices)
  * Remaining per-layer elementwise work, balanced across DVE and ACT:
      pair layers:  pull U' = n - pb  (DVE tensor_scalar from PSUM, or ACT
                    Identity from PSUM -- static split), q = U'*U' (DVE
                    tensor_tensor fp16 2x), T' = sqrt(q + ik) (ACT).
      ymat layers:  q = Square(n - pb) directly from PSUM (ACT),
                    T' = sqrt(q + ik) (ACT), Y' = cc*T' + n (DVE stt from
                    PSUM; carries offset pb which the host folds into the
                    next layer's biases).
  * Host casts fp32->fp16 for input/output transfers (halves HBM traffic).
"""

import numpy as np

BATCH = 65536
W = 512
HALF = 256
DEPTH = 32
NBITS = 9
NCORES = 8
NB = BATCH // NCORES          # batch rows per core
CH = 2048                     # batch columns per on-chip chunk
NTILE = 4                     # width tiles of 128 partitions
MMH = 512                     # moving free-dim per matmul (ISA cap)

CROSS = frozenset(l for l, d in enumerate(
    [1 << ((8 - l) % NBITS) for l in range(DEPTH)]) if d >= 128)
YMAT = frozenset(l for l in range(DEPTH) if (l + 1) in CROSS)

_P_ARR = np.array([(w >> 1) | ((w & 1) << 8) for w in range(W)], dtype=np.int64)


def _invert(p):
    inv = np.empty_like(p)
    inv[p] = np.arange(len(p))
    return inv


def _build_perms():
    pinv = _invert(_P_ARR)
    pis = [np.arange(W)]
    for l in range(DEPTH):
        pis.append(pinv[pis[l]])
    return pis


def _deltas():
    return [1 << ((8 - l) % NBITS) for l in range(DEPTH)]


def host_precompute(thetas, biases, slopes1, slopes2, curvatures):
    pis = _build_perms()
    thetas = thetas.astype(np.float64)
    c_all = np.cos(thetas)
    s_all = np.sin(thetas)
    m1 = np.exp(slopes1.astype(np.float64))
    m2 = np.exp(slopes2.astype(np.float64))
    a_all = (m1 + m2) * 0.5
    cc_all = (m2 - m1) / (2.0 * a_all)
    b_all = np.sinh(biases.astype(np.float64))
    ik_all = np.exp(-curvatures.astype(np.float64))

    idx_w = np.arange(W)
    layers = []
    ofs = np.zeros(W)                 # additive offset carried by Y tiles
    for l in range(DEPTH):
        pi_l, pi_n = pis[l], pis[l + 1]
        delta = 1 << ((8 - l) % NBITS)
        V = pi_l
        idx = V % HALF
        cA = c_all[l][idx]
        sA = np.where(V < HALF, s_all[l][idx], -s_all[l][idx])
        a_eff = a_all[l][pi_n]
        cAp = cA * a_eff
        sAp = sA * a_eff
        bb = b_all[l][pi_n]
        # effective pre-activation bias: subtract what the (offset) inputs
        # injected plus this layer's own bias
        pb = bb + cAp * ofs + sAp * ofs[idx_w ^ delta]
        ent = dict(delta=delta, cAp=cAp, sAp=sAp, pb=pb,
                   cc=cc_all[l][pi_n], ik=ik_all[l][pi_n])
        ofs = pb if l in YMAT else np.zeros(W)
        layers.append(ent)
    return layers, _invert(pis[DEPTH])


# ---- packing ---------------------------------------------------------------

def _wts_layout():
    out = []
    for l in range(DEPTH):
        for g in range(NTILE):
            if l in CROSS:
                out.append((l, g, "Ds"))
                out.append((l, g, "Dp"))
            else:
                out.append((l, g, "MU"))
                out.append((l, g, "MT"))
    return out


def pack_weights(layers):
    lay = _wts_layout()
    wts = np.zeros((128, len(lay) * 128), dtype=np.float16)
    p128 = np.arange(128)
    for j, (l, g, kind) in enumerate(lay):
        ent = layers[l]
        delta = ent["delta"]
        w = g * 128 + p128
        part = w ^ delta
        A = np.zeros((128, 128), dtype=np.float64)
        if kind == "Ds":
            A[p128, p128] = ent["cAp"][w]
        elif kind == "Dp":
            A[p128, p128] = ent["sAp"][w]
        elif kind == "MU":
            A[p128, p128] = ent["cAp"][w]
            A[p128, p128 ^ delta] = ent["sAp"][w]
        elif kind == "MT":
            prev = layers[l - 1]
            A[p128, p128] = ent["cAp"][w] * prev["cc"][w]
            A[p128, p128 ^ delta] = ent["sAp"][w] * prev["cc"][part]
        wts[:, j * 128:(j + 1) * 128] = A.T.astype(np.float16)
    return wts


_PKINDS = ("pb", "npb", "ik", "cc")
PRM_COLS = len(_PKINDS) * DEPTH * NTILE


def pack_params(layers):
    prm = np.zeros((128, PRM_COLS), dtype=np.float32)
    for l, ent in enumerate(layers):
        vals = dict(pb=ent["pb"], npb=-ent["pb"], ik=ent["ik"],
                    cc=ent["cc"])
        for k, kind in enumerate(_PKINDS):
            v = vals[kind]
            for g in range(NTILE):
                prm[:, (k * DEPTH + l) * NTILE + g] = v[g * 128:(g + 1) * 128]
    return prm


# ---- bass module -----------------------------------------------------------

def build_nc(nb=NB, ch=CH):
    from concourse import bacc, mybir
    from concourse.tile import TileContext

    f32 = mybir.dt.float32
    f16 = mybir.dt.float16
    AT = mybir.ActivationFunctionType
    OP = mybir.AluOpType

    deltas = _deltas()
    lay = _wts_layout()
    widx = {key: j for j, key in enumerate(lay)}

    nc = bacc.Bacc(target_bir_lowering=False, debug=False)
    xt = nc.declare_dram_parameter("xt", [W, nb], f16, isOutput=False)
    prm_d = nc.declare_dram_parameter("prm", [128, PRM_COLS], f32,
                                      isOutput=False)
    wts_d = nc.declare_dram_parameter("wts", [128, len(lay) * 128], f16,
                                      isOutput=False)
    out_d = nc.declare_dram_parameter("out", [W, nb], f16, isOutput=True)

    nchunk = nb // ch

    with TileContext(nc) as tc:
        with (
            tc.tile_pool(name="const", bufs=1) as cpool,
            tc.tile_pool(name="u", bufs=2) as upool,
            tc.tile_pool(name="t", bufs=2) as tpool,
            tc.tile_pool(name="q", bufs=3) as qpool,
            tc.tile_pool(name="y", bufs=2) as ypool,
            tc.tile_pool(name="ps", bufs=2, space="PSUM") as pspool,
        ):
            prm = cpool.tile([128, PRM_COLS], f32, tag="prm", name="prm")
            nc.sync.dma_start(out=prm, in_=prm_d[:, :])
            wsb = cpool.tile([128, len(lay) * 128], f16, tag="wts",
                             name="wsb")
            nw = len(lay) * 128
            for s in range(4):
                lo, hi = s * nw // 4, (s + 1) * nw // 4
                nc.sync.dma_start(out=wsb[:, lo:hi], in_=wts_d[:, lo:hi])

            def pcol(kind, l, g):
                k = _PKINDS.index(kind)
                c = (k * DEPTH + l) * NTILE + g
                return prm[:, c:c + 1]

            def lhs(l, g, kind):
                j = widx[(l, g, kind)]
                return wsb[:, j * 128:(j + 1) * 128]

            xt_r = xt.rearrange("(g p) b -> p g b", g=NTILE)
            out_r = out_d.rearrange("(g p) b -> p g b", g=NTILE)

            for c in range(nchunk):
                # layer-0 inputs are Y tiles (offset 0)
                Y = {}
                for g in range(NTILE):
                    Y[g] = ypool.tile([128, ch], f16, tag=f"y{g}",
                                      name=f"y{g}")
                    nc.sync.dma_start(out=Y[g],
                                      in_=xt_r[:, g, c * ch:(c + 1) * ch])

                U = {}
                T = {}
                for l in range(DEPTH):
                    delta = deltas[l]
                    Un, Tn, Yn = {}, {}, {}
                    for g in range(NTILE):
                        ps = pspool.tile([128, ch], f32, tag="ps", name="ps")
                        for h in range(0, ch, MMH):
                            sl = slice(h, h + MMH)
                            if l in CROSS:
                                gp = g ^ (delta >> 7)
                                nc.tensor.matmul(
                                    ps[:, sl], lhs(l, g, "Ds"), Y[g][:, sl],
                                    start=True, stop=False)
                                nc.tensor.matmul(
                                    ps[:, sl], lhs(l, g, "Dp"), Y[gp][:, sl],
                                    start=False, stop=True)
                            else:
                                nc.tensor.matmul(
                                    ps[:, sl], lhs(l, g, "MU"), U[g][:, sl],
                                    start=True, stop=False)
                                nc.tensor.matmul(
                                    ps[:, sl], lhs(l, g, "MT"), T[g][:, sl],
                                    start=False, stop=True)
                        if l in YMAT:
                            # q = (n - pb)^2 straight from PSUM on ACT
                            q = qpool.tile([128, ch], f16, tag="q", name="q")
                            nc.scalar.activation(
                                q, ps, AT.Square, bias=pcol("npb", l, g),
                                scale=1.0)
                            t = tpool.tile([128, ch], f16, tag=f"t{g}",
                                           name=f"t{g}")
                            nc.scalar.activation(
                                t, q, AT.Sqrt, bias=pcol("ik", l, g),
                                scale=1.0)
                            y = ypool.tile([128, ch], f16, tag=f"y{g}",
                                           name=f"y{g}")
                            nc.vector.scalar_tensor_tensor(
                                y, t, pcol("cc", l, g), ps, OP.mult, OP.add)
                            Yn[g] = y
                        else:
                            u = upool.tile([128, ch], f16, tag=f"u{g}",
                                           name=f"u{g}")
                            if (l * NTILE + g + c) % 8 < 7:  # ~87% DVE pulls
                                nc.vector.tensor_scalar(
                                    u, ps, pcol("pb", l, g), None,
                                    OP.subtract)
                            else:
                                nc.scalar.activation(
                                    u, ps, AT.Identity,
                                    bias=pcol("npb", l, g), scale=1.0)
                            q = qpool.tile([128, ch], f16, tag="q", name="q")
                            if (l + g + c) % 2 == 0:  # half squares on GpSimd
                                nc.gpsimd.tensor_tensor(q, u, u, OP.mult)
                            else:
                                nc.vector.tensor_tensor(q, u, u, OP.mult)
                            t = tpool.tile([128, ch], f16, tag=f"t{g}",
                                           name=f"t{g}")
                            nc.scalar.activation(
                                t, q, AT.Sqrt, bias=pcol("ik", l, g),
                                scale=1.0)
                            Un[g] = u
                            Tn[g] = t
                    if Yn:
                        Y = Yn
                    U, T = (Un or U), (Tn or T)

                for g in range(NTILE):
                    y = ypool.tile([128, ch], f16, tag=f"y{g}", name=f"o{g}")
                    nc.vector.scalar_tensor_tensor(
                        y, T[g], pcol("cc", DEPTH - 1, g), U[g],
                        OP.mult, OP.add)
                    nc.sync.dma_start(
                        out=out_r[:, g, c * ch:(c + 1) * ch], in_=y)
    nc.compile()
    return nc


_NC_CACHE = {}

TRACE = False
TRACE_KWARGS = {}
LAST_RESULTS = None


def _get_nc(nb, ch):
    key = (nb, ch)
    if key not in _NC_CACHE:
        _NC_CACHE[key] = build_nc(nb, ch)
    return _NC_CACHE[key]


def kernel(X, thetas, biases, slopes1, slopes2, curvatures):
    global LAST_RESULTS
    from concourse.bass_utils import run_bass_kernel_spmd

    X = np.asarray(X)
    layers, out_perm = host_precompute(
        np.asarray(thetas), np.asarray(biases), np.asarray(slopes1),
        np.asarray(slopes2), np.asarray(curvatures))
    prm = pack_params(layers)
    wts = pack_weights(layers)

    nc = _get_nc(NB, CH)
    in_maps = []
    for cid in range(NCORES):
        shard = np.ascontiguousarray(
            X[cid * NB:(cid + 1) * NB, :].T.astype(np.float16))
        in_maps.append({"xt": shard, "prm": prm, "wts": wts})

    res = run_bass_kernel_spmd(nc, in_maps, list(range(NCORES)),
                               trace=TRACE, **TRACE_KWARGS)
    LAST_RESULTS = res
    out = np.empty((BATCH, W), dtype=np.float32)
    for cid in range(NCORES):
        o = res.results[cid]["out"]          # [512, NB] fp16 physical order
        out[cid * NB:(cid + 1) * NB, :] = o[out_perm, :].T.astype(np.float32)
    return out
